# revision 9
# baseline (speedup 1.0000x reference)
"""Low-rank layer y = (U^T V) @ x computed as y = U^T @ (V @ x).

Full problem: x [8192, 4096] f32, U/V [8, 8192] f32, y [8192, 4096] f32.
Sharding: batch (columns of x) split across 8 NeuronCores, 512 per core.

Device I/O is bf16 (host quantizes x/U/V, upcasts y), halving HBM traffic
to 16 MiB per core; rel err ~5e-3 vs the 2e-2 gate.

Each core's 512 batch columns are further split into two halves of 256
that are software-pipelined: while half B's input stream is still on the
DMA engines, half A's phase 2 (y = U^T @ T_A) runs on PE/ACT/DVE and its
stores queue up behind the input stream, so the DMA engines flow from
input directly into output with no phase-transition bubble. Store order
on the single per-core DMA resource is: x_A stream, vt/u, x_B stream,
y_A stores, y_B stores — gapless except for launch latency.

Layouts are packed host-side so every DMA is a plain 2D slice with
multi-KiB contiguous per-partition runs:
  xp[p, h*16384 + n*256 + b] = x[n*128 + p, core*512 + h*256 + b]
and yp identically for the output.
"""

import numpy as np

L = 8192
RANK = 8
BATCH = 4096
NCORES = 8
BS = BATCH // NCORES  # 512 batch columns per core
HB = BS // 2          # 256-column half processed as one pipeline unit
P = 128               # SBUF partitions
NCHUNK = L // P       # 64 row-chunks of 128
XG = 8                # L-chunks per input DMA group
NXG = NCHUNK // XG    # 8 input groups per half
YG = 8                # L-chunks per output stage
NYS = NCHUNK // YG    # 8 output stages per half

_NC = None  # cached compiled Bass module


def _body(tc, nc, xp, vt, u, yp, mybir):
    from contextlib import ExitStack

    f32 = mybir.dt.float32
    bf16 = mybir.dt.bfloat16
    HC = NCHUNK * HB  # columns per half in xp/yp (16384)

    with ExitStack() as ctx:
        const = ctx.enter_context(tc.tile_pool(name="const", bufs=1))
        xpool = ctx.enter_context(tc.tile_pool(name="xbuf", bufs=1))
        tpsum = ctx.enter_context(tc.tile_pool(name="tpsum", bufs=1, space="PSUM"))
        ypsum = ctx.enter_context(tc.tile_pool(name="ypsum", bufs=6, space="PSUM"))
        ybuf = ctx.enter_context(tc.tile_pool(name="ybuf", bufs=1))

        # Input DMAs for half A, group 0 first so the stream owns the DMA
        # engines immediately; the tiny vt/u transfers slot in after group 1.
        xa = []
        for d in range(2):
            xt = xpool.tile([P, XG * HB], bf16, tag=f"xa{d}")
            nc.sync.dma_start(xt[:], xp[:, d * XG * HB:(d + 1) * XG * HB])
            xa.append(xt)

        vt_sb = const.tile([P, NCHUNK * RANK], bf16)  # vt[p, n*8+r] = V[r, n*128+p]
        nc.sync.dma_start(vt_sb[:], vt[:])
        u_sb = const.tile([RANK, L], bf16)
        nc.sync.dma_start(u_sb[:], u[:])

        for d in range(2, NXG):
            xt = xpool.tile([P, XG * HB], bf16, tag=f"xa{d}")
            nc.sync.dma_start(xt[:], xp[:, d * XG * HB:(d + 1) * XG * HB])
            xa.append(xt)
        xb = []
        for d in range(NXG):
            xt = xpool.tile([P, XG * HB], bf16, tag=f"xb{d}")
            nc.sync.dma_start(xt[:], xp[:, HC + d * XG * HB:HC + (d + 1) * XG * HB])
            xb.append(xt)

        t_sb_a = const.tile([RANK, HB], bf16)
        t_sb_b = const.tile([RANK, HB], bf16)
        y_sb = ybuf.tile([P, 2 * HC], bf16)

        # Dummy matmul reading ONLY vt_sb: absorbs the vt DMA wait so the
        # first real matmul carries a single sync wait. Shares the t_ps PSUM
        # bank (same tag); same-engine ordering serializes the reuse.
        warm1 = tpsum.tile([RANK, RANK], f32, tag="tps")
        nc.tensor.matmul(warm1[:], vt_sb[:, 0:RANK], vt_sb[:, 0:RANK],
                         start=True, stop=True)

        def p1_group(t_ps, xtile, d):
            for c in range(XG):
                n = d * XG + c
                nc.tensor.matmul(
                    t_ps[:],
                    vt_sb[:, n * RANK:(n + 1) * RANK],  # lhsT [128, 8]
                    xtile[:, c * HB:(c + 1) * HB],      # rhs  [128, 256]
                    start=(n == 0),
                    stop=(n == NCHUNK - 1),
                )

        ncopy = 0

        def p2_pair(t_sb, h, k):
            # y chunks 2k, 2k+1 of half h: two 256-col matmuls into one
            # PSUM bank, drained by one 512-col copy (alternating ACT/DVE).
            nonlocal ncopy
            y_ps = ypsum.tile([P, 2 * HB], f32, tag="yp")
            for j in range(2):
                n = 2 * k + j
                nc.tensor.matmul(
                    y_ps[:, j * HB:(j + 1) * HB],
                    u_sb[:, n * P:(n + 1) * P],  # lhsT [8, 128]
                    t_sb[:],                     # rhs  [8, 256]
                    start=True,
                    stop=True,
                )
            lo = h * HC + 2 * k * HB
            if ncopy % 2 == 0:
                nc.scalar.copy(y_sb[:, lo:lo + 2 * HB], y_ps[:])
            else:
                nc.vector.tensor_copy(y_sb[:, lo:lo + 2 * HB], y_ps[:])
            ncopy += 1

        def store_stage(h, s):
            lo = h * HC + s * YG * HB
            # SWDGE (gpsimd): fresh DMASW sem lanes, no HWDGE lane-recycle
            # waits pile onto these instructions.
            nc.gpsimd.dma_start(yp[:, lo:lo + YG * HB], y_sb[:, lo:lo + YG * HB])

        # Half A phase 1: T_A = V @ x_A accumulated in PSUM.
        t_ps_a = tpsum.tile([RANK, HB], f32, tag="tps")
        for d in range(NXG):
            p1_group(t_ps_a, xa[d], d)
        nc.vector.tensor_copy(t_sb_a[:], t_ps_a[:])

        # Rounds: half B phase-1 group d, then half A phase-2 stage d
        # (4 pairs + its store). B's input stream paces PE; A's phase-2
        # output work fills the gaps and queues stores behind the stream.
        t_ps_b = tpsum.tile([RANK, HB], f32, tag="tps")
        for d in range(NXG):
            p1_group(t_ps_b, xb[d], d)
            for k in range(4 * d, 4 * d + 4):
                p2_pair(t_sb_a, 0, k)
            store_stage(0, d)
        nc.vector.tensor_copy(t_sb_b[:], t_ps_b[:])

        # Half B phase 2.
        for s in range(NYS):
            for k in range(4 * s, 4 * s + 4):
                p2_pair(t_sb_b, 1, k)
            store_stage(1, s)


def build_bass():
    import concourse.mybir as mybir
    import concourse.tile as tile
    from concourse import bacc

    # Bacc (not raw Bass): its compile() runs generate_event_semaphores(),
    # which splits multi-sem waits into the 1-wait-per-instruction form the
    # TRN2 ISA requires.
    nc = bacc.Bacc("TRN2", target_bir_lowering=False, debug=False)
    bf16 = mybir.dt.bfloat16
    xp = nc.dram_tensor("xp", [P, NCHUNK * BS], bf16, kind="ExternalInput").ap()
    vt = nc.dram_tensor("vt", [P, NCHUNK * RANK], bf16, kind="ExternalInput").ap()
    u = nc.dram_tensor("u", [RANK, L], bf16, kind="ExternalInput").ap()
    yp = nc.dram_tensor("yp", [P, NCHUNK * BS], bf16, kind="ExternalOutput").ap()

    with tile.TileContext(nc) as tc:
        _body(tc, nc, xp, vt, u, yp, mybir)
    nc.compile()
    return nc


def _get_nc():
    global _NC
    if _NC is None:
        _NC = build_bass()
    return _NC


def make_in_maps(inputs, U, V):
    import ml_dtypes

    bf16 = ml_dtypes.bfloat16
    x = np.asarray(inputs, dtype=np.float32).astype(bf16)
    # xp[p, h*16384 + n*HB + b] = x[n*128 + p, c*512 + h*256 + b]
    x5 = x.reshape(NCHUNK, P, NCORES, 2, HB).transpose(2, 3, 1, 0, 4)  # [c,h,p,n,b]
    # vt[p, n*RANK + r] = V[r, n*128 + p]
    vt = np.ascontiguousarray(
        np.asarray(V, dtype=np.float32).astype(bf16)
        .reshape(RANK, NCHUNK, P).transpose(2, 1, 0).reshape(P, NCHUNK * RANK)
    )
    u = np.ascontiguousarray(np.asarray(U, dtype=np.float32).astype(bf16))
    in_maps = []
    for c in range(NCORES):
        xpc = np.ascontiguousarray(
            x5[c].transpose(1, 0, 2, 3).reshape(P, NCHUNK * BS))
        in_maps.append({"xp": xpc, "vt": vt, "u": u})
    return in_maps


def finish(res, inputs_np=None):
    # yp[p, h*16384 + n*HB + b] -> y[n*128 + p, c*512 + h*256 + b]
    cols = []
    for c in range(NCORES):
        ypc = np.asarray(res.results[c]["yp"]).reshape(P, 2, NCHUNK, HB)
        cols.append(
            ypc.transpose(2, 0, 1, 3).reshape(L, BS).astype(np.float32))
    return np.concatenate(cols, axis=1)


def kernel(inputs, U, V):
    from concourse import bass_utils

    nc = _get_nc()
    in_maps = make_in_maps(inputs, U, V)
    res = bass_utils.run_bass_kernel_spmd(nc, in_maps, core_ids=list(range(NCORES)))
    return finish(res)
